# revision 2
# baseline (speedup 1.0000x reference)
"""Trainium2 Bass kernel v2 for nn_LocalAttention (sparse point-cloud attn).

Structure (per core, SPMD x8; full N=50176 table replicated per core):
  stage 0: consts + Q-projection (own shard) + rel-MLP bias precompute
           (bias depends only on pos/idx -> all 49 tiles hoisted, kept in
           SBUF as bias_all; rel comes pre-transposed from host).
  stage 1 (phase A): K/V projections for ALL 50176 points, written to a
           DRAM pair-table: row j = [K(2j)|V(2j)|K(2j+1)|V(2j+1)], fp8 or
           bf16 halves; 4 g2-blocks (1024 points) per DMA -> 49 loads +
           49 stores instead of 392 (per-DMA overhead ~2us dominates).
  stage 2: per tile ONE dma_gather (custom Q7 instruction, 2048 pair rows,
           single_packet=False, 4 SWDGE queues round-robin) -> parity
           select via bitwise copy_predicated -> attention on DVE ->
           out-proj/LN; xres loads + y stores batched by 7 tiles.

The pair-table halves the index range so pair indices fit the gather's
int16 (25088 < 32768); the 2x fetch is the price, paid at ~210 GB/s.
"""

import sys

import numpy as np

for _p in ("/opt/trn_rl_repo",):
    if _p not in sys.path:
        sys.path.insert(0, _p)

import ml_dtypes

import concourse.bass as bass
import concourse.tile as tile
from concourse import bacc, mybir
from concourse.masks import make_identity

BF16 = mybir.dt.bfloat16
F32 = mybir.dt.float32
F8 = mybir.dt.float8e4
I32 = mybir.dt.int32
I16 = mybir.dt.int16
U16 = mybir.dt.uint16

N, C, H, D, K = 50000, 256, 8, 8 * 4, 16
SCALE = D**-0.5
EPS = 1e-5
NCORES = 8
P = 128

V_FP8 = True  # V half dtype in the table (K is always fp8)

KB = 128          # K lanes (bf16 lanes; 256 fp8 bytes)
VB = 128 if V_FP8 else 256   # V lanes
BL = KB + VB      # point-block lanes
GE = 2 * BL       # pair-row lanes (gather elem_size)

nbf16 = ml_dtypes.bfloat16


def full_cfg():
    SH = 6272
    return dict(NPAD=NCORES * SH, SH=SH, T=SH // P, G=NCORES * SH // P,
                ncores=NCORES)


def host_prep(inputs, cfg):
    NPAD, SH, T, G = cfg["NPAD"], cfg["SH"], cfg["T"], cfg["G"]
    ncores = cfg["ncores"]
    n = inputs["x"].shape[0]

    x = np.asarray(inputs["x"], np.float32)
    pos = np.asarray(inputs["pos"], np.float32)
    idx = np.asarray(inputs["idx"]).astype(np.int32)
    Wq = np.asarray(inputs["Wq"], np.float32)
    bq = np.asarray(inputs["bq"], np.float32)
    Wk = np.asarray(inputs["Wk"], np.float32)
    Wv = np.asarray(inputs["Wv"], np.float32)
    Wo = np.asarray(inputs["Wo"], np.float32)
    bo = np.asarray(inputs["bo"], np.float32)
    bv = np.asarray(inputs["bv"], np.float32)
    W1 = np.asarray(inputs["W1"], np.float32)
    b1 = np.asarray(inputs["b1"], np.float32)
    W2 = np.asarray(inputs["W2"], np.float32)
    gamma = np.asarray(inputs["gamma"], np.float32)
    beta = np.asarray(inputs["beta"], np.float32)

    xpad = np.zeros((NPAD, C), np.float32)
    xpad[:n] = x
    pospad = np.zeros((NPAD, 3), np.float32)
    pospad[:n] = pos
    idxpad = np.zeros((NPAD, K), np.int32)
    idxpad[:n] = idx

    a = xpad.reshape(G, P, 2, P)
    xtt = np.ascontiguousarray(a.transpose(0, 3, 2, 1)).astype(nbf16)
    xtt2 = np.ascontiguousarray(
        xtt.reshape(G // 2, 2, P, 2, P).transpose(0, 2, 1, 3, 4)
    ).reshape(G // 2, P, 4, P)

    def wlay(W):
        return np.ascontiguousarray(
            W.reshape(2, P, C).transpose(1, 0, 2)).astype(nbf16)

    wq_l = wlay(Wq * SCALE)
    wkv = np.concatenate([Wk, Wv], 1)          # [C, 512]
    wkv_l = np.ascontiguousarray(
        wkv.reshape(2, P, 512).transpose(1, 0, 2)).astype(nbf16)
    wo_l = wlay(Wo)
    w1aug = np.concatenate([W1, b1[None, :]], 0).astype(nbf16)  # [4, 64]
    w2s = W2.astype(nbf16)                                       # [64, H]
    bq_s = (bq * SCALE).astype(np.float32)
    bo_eff = (bv @ Wo + bo).astype(np.float32)

    shared = dict(xtt=xtt2, wq=wq_l, wkv=wkv_l, wo=wo_l, w1a=w1aug,
                  w2s=w2s, bq=bq_s, boeff=bo_eff, gamma=gamma, beta=beta)

    in_maps = []
    for c in range(ncores):
        lo = c * SH
        sl = slice(lo, lo + SH)
        idc = idxpad[sl]                                # [SH, K] int32
        # pair index (int16) in gather wrap layout: list pos i=k*128+p
        pidx = (idc >> 1).astype(np.int16).reshape(T, P, K)
        i16 = np.zeros((T, P, P), np.int16)
        for t in range(T):
            lst = pidx[t].T.ravel()                     # [2048] : k*128+p
            i16[t] = np.tile(lst.reshape(P, 16).T, (8, 1))
        idx16 = np.ascontiguousarray(
            i16.transpose(1, 0, 2)).reshape(P, T * P)
        # parity mask (uint16 0/1), natural [p, t*K+k]
        pmask = np.ascontiguousarray(
            (idc & 1).astype(np.uint16).reshape(T, P, K).transpose(1, 0, 2)
        ).reshape(P, T * K)
        # rel (pos[idx]-pos) + ones row, transposed to [4, T*K*128]:
        # relt4[a, t*2048 + k*128 + p]
        rel = pospad[idc] - pospad[sl][:, None, :]      # [SH, K, 3]
        r4 = rel.reshape(T, P, K, 3)
        relTall = np.ones((4, T * K * P), np.float32)
        relTall[:3] = np.ascontiguousarray(
            r4.transpose(3, 0, 2, 1)).reshape(3, T * K * P)
        relTall = relTall.astype(nbf16)

        m = dict(shared)
        m.update(xres=np.ascontiguousarray(xpad[sl]),
                 xqt=np.ascontiguousarray(xtt[c * T:(c + 1) * T]),
                 idx16=idx16, pmask=pmask, relt=relTall)
        in_maps.append(m)
    return in_maps


def build_nc(cfg, variant="full"):
    NPAD, SH, T, G = cfg["NPAD"], cfg["SH"], cfg["T"], cfg["G"]
    PAIRS = NPAD // 2

    nc = bacc.Bacc(trn_type="TRN2", num_swdge_queues=4)

    xtt = nc.dram_tensor("xtt", [G // 2, P, 4, P], BF16, kind="ExternalInput")
    xqt = nc.dram_tensor("xqt", [T, P, 2, P], BF16, kind="ExternalInput")
    xres = nc.dram_tensor("xres", [SH, C], F32, kind="ExternalInput")
    idx16 = nc.dram_tensor("idx16", [P, T * P], I16, kind="ExternalInput")
    pmask = nc.dram_tensor("pmask", [P, T * K], U16, kind="ExternalInput")
    relt = nc.dram_tensor("relt", [4, T * K * P], BF16, kind="ExternalInput")
    wq = nc.dram_tensor("wq", [P, 2, C], BF16, kind="ExternalInput")
    wkv = nc.dram_tensor("wkv", [P, 2, 512], BF16, kind="ExternalInput")
    wo = nc.dram_tensor("wo", [P, 2, C], BF16, kind="ExternalInput")
    w1a = nc.dram_tensor("w1a", [4, 64], BF16, kind="ExternalInput")
    w2s = nc.dram_tensor("w2s", [64, H], BF16, kind="ExternalInput")
    bq = nc.dram_tensor("bq", [C], F32, kind="ExternalInput")
    boeff = nc.dram_tensor("boeff", [C], F32, kind="ExternalInput")
    gamma = nc.dram_tensor("gamma", [C], F32, kind="ExternalInput")
    beta = nc.dram_tensor("beta", [C], F32, kind="ExternalInput")
    y = nc.dram_tensor("y", [SH, C], F32, kind="ExternalOutput")

    xtab = nc.dram_tensor("xtab", [PAIRS, GE], BF16)

    def bcast_vec(v, cols):
        return bass.AP(tensor=v.ap().tensor, offset=0, ap=[[0, P], [1, cols]])

    do_s0 = variant in ("full", "s01", "s0")
    do_s1 = variant in ("full", "s01")
    do_s2g = variant in ("full", "s2g")
    do_s2c = variant in ("full", "s2c")

    with tile.TileContext(nc) as tc:
        import contextlib

        with contextlib.ExitStack() as ctx:
            consts = ctx.enter_context(tc.tile_pool(name="consts", bufs=1))

            wq_sb = consts.tile([P, 2, C], BF16)
            wkv_sb = consts.tile([P, 2, 512], BF16)
            wo_sb = consts.tile([P, 2, C], BF16)
            for t_sb, t_dr in ((wq_sb, wq), (wkv_sb, wkv), (wo_sb, wo)):
                nc.sync.dma_start(out=t_sb[:], in_=t_dr[:, :, :])
            w1_sb = consts.tile([4, 64], BF16)
            nc.sync.dma_start(out=w1_sb[:], in_=w1a[:, :])
            w2_sb = consts.tile([64, H], BF16)
            nc.sync.dma_start(out=w2_sb[:], in_=w2s[:, :])
            idx_sb = consts.tile([P, T * P], I16)
            nc.sync.dma_start(out=idx_sb[:], in_=idx16[:, :])
            msk_sb = consts.tile([P, T * K], U16)
            nc.sync.dma_start(out=msk_sb[:], in_=pmask[:, :])
            bq_sb = consts.tile([P, C], F32)
            nc.sync.dma_start(out=bq_sb[:], in_=bcast_vec(bq, C))
            bo_sb = consts.tile([P, C], F32)
            nc.sync.dma_start(out=bo_sb[:], in_=bcast_vec(boeff, C))
            gam_sb = consts.tile([P, C], F32)
            nc.sync.dma_start(out=gam_sb[:], in_=bcast_vec(gamma, C))
            bet_sb = consts.tile([P, C], F32)
            nc.sync.dma_start(out=bet_sb[:], in_=bcast_vec(beta, C))
            eps_sb = consts.tile([P, 1], F32)
            nc.vector.memset(eps_sb[:], EPS)
            invc_sb = consts.tile([P, 1], F32)
            nc.vector.memset(invc_sb[:], 1.0 / C)
            ident = consts.tile([P, P], F32)
            make_identity(nc, ident[:])
            ident_bf = consts.tile([P, P], BF16)
            nc.vector.tensor_copy(out=ident_bf[:], in_=ident[:])

            q_all = consts.tile([P, T, C], BF16)
            bias_all = consts.tile([P, T, K, H], BF16)

            zero_sb = consts.tile([P, C], F32)
            nc.vector.memset(zero_sb[:], 0.0)

            # ---- stage 1: phase A (K/V table build), merged K|V matmul,
            # one fp8 copy per point-tile, loads on the SWDGE mainline ring
            NBA = 4
            if not do_s1:
                pass
            else:
              with tc.tile_pool(name="pa", bufs=3) as pa, \
                    tc.tile_pool(name="paps", bufs=2, space="PSUM") as paps:
                for gb in range(G // 2 // NBA):
                    xt = pa.tile([P, NBA, 4, P], BF16, tag="xt")
                    src = bass.AP(tensor=xtt.ap().tensor,
                                  offset=gb * NBA * P * 4 * P,
                                  ap=[[4 * P, P], [P * 4 * P, NBA],
                                      [P, 4], [1, P]])
                    nc.sync.dma_start(out=xt[:], in_=src)
                    row = pa.tile([P, 2 * NBA, BL], BF16, tag="row")
                    for bo_ in range(NBA):
                        for pr in range(2):
                            r = 2 * bo_ + pr
                            kvps = paps.tile([P, 512], F32, tag="kvps")
                            for j in range(2):
                                nc.tensor.matmul(kvps[:],
                                                 lhsT=xt[:, bo_, pr * 2 + j, :],
                                                 rhs=wkv_sb[:, j, :],
                                                 start=(j == 0), stop=(j == 1))
                            if r % 2 == 0:
                                nc.scalar.copy(
                                    out=row[:, r, 0:BL].bitcast(F8),
                                    in_=kvps[:])
                            else:
                                nc.vector.tensor_copy(
                                    out=row[:, r, 0:BL].bitcast(F8),
                                    in_=kvps[:])
                    # store: point (g*128+p) -> pair g*64+p//2, parity p%2
                    tout = bass.AP(tensor=xtab.ap().tensor,
                                   offset=gb * 2 * NBA * 64 * GE,
                                   ap=[[GE, 64], [BL, 2],
                                       [64 * GE, 2 * NBA], [1, BL]])
                    nc.sync.dma_start(out=tout, in_=row[:])

            # ---- stage 0a: Q projections (own shard), 7-tile batched loads
            if not do_s0:
                pass
            else:
              with tc.tile_pool(name="pq", bufs=3) as pq, \
                    tc.tile_pool(name="pqps", bufs=2, space="PSUM") as pqps:
                for tb in range(T // 7):
                    xq = pq.tile([P, 7, 2, P], BF16, tag="xq")
                    src = bass.AP(tensor=xqt.ap().tensor,
                                  offset=tb * 7 * P * 2 * P,
                                  ap=[[2 * P, P], [P * 2 * P, 7],
                                      [P, 2], [1, P]])
                    nc.sync.dma_start(out=xq[:], in_=src)
                    for ti in range(7):
                        t = tb * 7 + ti
                        qps = pqps.tile([P, C], F32, tag="qps")
                        for j in range(2):
                            nc.tensor.matmul(qps[:], lhsT=xq[:, ti, j, :],
                                             rhs=wq_sb[:, j, :],
                                             start=(j == 0), stop=(j == 1))
                        nc.vector.tensor_tensor(out=q_all[:, t, :],
                                                in0=qps[:], in1=bq_sb[:],
                                                op=mybir.AluOpType.add)

            # ---- stage 0b: rel-MLP bias, transpose-free.
            # hidT[j, k*128+p] = sum_a w1aug[a, j] * relT4[a, k*128+p]
            # (contract over 4 partitions); gelu; then per k-block of 128
            # cols: bias[p, k, :] = hidT_blk^T @ W2 (contract over 64).
            if not do_s0:
                pass
            else:
              with tc.tile_pool(name="pm", bufs=4) as pm, \
                    tc.tile_pool(name="pmh", bufs=2, space="PSUM") as pmh, \
                    tc.tile_pool(name="pmb", bufs=2, space="PSUM") as pmb:
                for t2 in range(T // 2 + 1):
                    tlist = ([2 * t2, 2 * t2 + 1] if 2 * t2 + 1 < T
                             else [2 * t2])
                    rl = pm.tile([4, 2, 2048], BF16, tag="rl")
                    nc.sync.dma_start(
                        out=rl[:, 0:len(tlist), :],
                        in_=relt[:, tlist[0] * 2048:
                                 (tlist[-1] + 1) * 2048].rearrange(
                            "a (b c) -> a b c", c=2048))
                    for li, t in enumerate(tlist):
                        bias_ps = pmb.tile([P, K, H], F32, tag="bias")
                        for hh in range(2):
                            hp = pmh.tile([64, 2, 512], F32, tag="hid")
                            for ch in range(2):
                                cc = hh * 2 + ch
                                nc.tensor.matmul(
                                    hp[:, ch, :], lhsT=w1_sb[:],
                                    rhs=rl[:, li, cc * 512:(cc + 1) * 512],
                                    start=True, stop=True)
                            hsb = pm.tile([64, 2, 512], BF16, tag="hsb")
                            nc.scalar.activation(
                                out=hsb[:], in_=hp[:],
                                func=mybir.ActivationFunctionType.Gelu)
                            h2 = hsb[:].rearrange("a b c -> a (b c)")
                            for kb in range(8):
                                k = hh * 8 + kb
                                nc.tensor.matmul(
                                    bias_ps[:, k, :],
                                    lhsT=h2[:, kb * P:(kb + 1) * P],
                                    rhs=w2_sb[:], start=True, stop=True)
                        nc.vector.tensor_copy(out=bias_all[:, t, :, :],
                                              in_=bias_ps[:])

            # ---- stage 2: gather + attention; 7-tile batched tail
            with contextlib.ExitStack() as bctx:
                pb = bctx.enter_context(tc.tile_pool(name="pb", bufs=2))
                pgt = bctx.enter_context(tc.tile_pool(name="pgt", bufs=3))
                pkv = bctx.enter_context(tc.tile_pool(name="pkv", bufs=2))
                pio = bctx.enter_context(tc.tile_pool(name="pio", bufs=2))
                py0 = bctx.enter_context(tc.tile_pool(name="py0", bufs=1))
                psm = bctx.enter_context(tc.tile_pool(name="psm", bufs=2))
                ps_mm = bctx.enter_context(
                    tc.tile_pool(name="ps_mm", bufs=1, space="PSUM"))
                ps_tp = bctx.enter_context(
                    tc.tile_pool(name="ps_tp", bufs=1, space="PSUM"))
                ps_l = bctx.enter_context(
                    tc.tile_pool(name="ps_l", bufs=2, space="PSUM"))

                if not (do_s2c or do_s2g):
                    for t in range(T):
                        nc.sync.dma_start(out=y[t * P:(t + 1) * P, :],
                                          in_=zero_sb[:])
                if do_s2c and not do_s2g:
                    kv_static = pgt.tile([P, K, GE], BF16, tag="kvst")
                    nc.vector.memset(
                        kv_static[:].rearrange("p a b -> p (a b)"), 0.25)

                NB2 = 7
                for b in range(T // NB2 if (do_s2c or do_s2g) else 0):
                    if do_s2c:
                        xr7 = pio.tile([P, NB2, C], F32, tag="xr7")
                        src = bass.AP(tensor=xres.ap().tensor,
                                      offset=b * NB2 * P * C,
                                      ap=[[C, P], [P * C, NB2], [1, C]])
                        nc.sync.dma_start(out=xr7[:], in_=src)
                        yb7 = pio.tile([P, NB2, C], F32, tag="yb7")
                        avsb7 = pb.tile([P, NB2, C], BF16, tag="avsb7")

                    for ti in range(NB2):
                        t = b * NB2 + ti
                        if do_s2g:
                            kv = pgt.tile([P, K, GE], BF16, tag="kv")
                            nc.gpsimd.dma_gather(
                                kv[:], xtab[:, :],
                                idx_sb[:, t * P:(t + 1) * P],
                                2048, 2048, GE, queue_num=t % 4,
                                single_packet=False)
                        else:
                            kv = kv_static
                        if not do_s2c:
                            nc.sync.dma_start(out=y[t * P:(t + 1) * P, :],
                                              in_=zero_sb[:])
                            continue

                        # parity select (bit-exact on uint16 lanes)
                        kvs = pkv.tile([P, K, BL], BF16, tag="kvs")
                        kvs_u = kvs[:].bitcast(U16)
                        kv_u = kv[:].bitcast(U16)
                        nc.vector.tensor_copy(out=kvs_u,
                                              in_=kv_u[:, :, 0:BL])
                        msl = msk_sb[:, t * K:(t + 1) * K]
                        mb_ap = bass.AP(tensor=msl.tensor, offset=msl.offset,
                                        ap=[msl.ap[0], [1, K], [0, BL]])
                        nc.vector.copy_predicated(out=kvs_u, mask=mb_ap,
                                                  data=kv_u[:, :, BL:2 * BL])

                        q_sb = q_all[:, t, :]
                        kf8 = kvs[:, :, 0:KB].bitcast(F8)
                        kf = bass.AP(tensor=kf8.tensor, offset=kf8.offset,
                                     ap=[kf8.ap[0], [2 * BL, K],
                                         [D, H], [1, D]])
                        qb = bass.AP(tensor=q_sb.tensor, offset=q_sb.offset,
                                     ap=[q_sb.ap[0], [0, K], [D, H], [1, D]])
                        work = pkv.tile([P, K, C], BF16, tag="work")
                        prod = work[:].rearrange("p s (h d) -> p s h d", h=H)
                        nc.vector.tensor_tensor(out=prod, in0=kf, in1=qb,
                                                op=mybir.AluOpType.mult)
                        logits = ps_l.tile([P, K, H], F32, tag="log")
                        nc.vector.tensor_reduce(out=logits[:], in_=prod,
                                                axis=mybir.AxisListType.X,
                                                op=mybir.AluOpType.add)
                        nc.vector.tensor_tensor(out=logits[:], in0=logits[:],
                                                in1=bias_all[:, t, :, :],
                                                op=mybir.AluOpType.add)

                        pex = psm.tile([P, K, H], F32, tag="pex")
                        nc.scalar.activation(
                            out=pex[:], in_=logits[:],
                            func=mybir.ActivationFunctionType.Exp)
                        ssum = psm.tile([P, H], F32, tag="ssum")
                        pex_hk = bass.AP(tensor=pex.tensor,
                                         offset=pex[:].offset,
                                         ap=[pex[:].ap[0], [1, H], [H, K]])
                        nc.vector.tensor_reduce(out=ssum[:], in_=pex_hk,
                                                axis=mybir.AxisListType.X,
                                                op=mybir.AluOpType.add)
                        rinv = psm.tile([P, H], F32, tag="rinv")
                        nc.vector.reciprocal(out=rinv[:], in_=ssum[:])
                        attn = pb.tile([P, K, H], BF16, tag="attn")
                        rib = bass.AP(tensor=rinv.tensor,
                                      offset=rinv[:].offset,
                                      ap=[rinv[:].ap[0], [0, K], [1, H]])
                        nc.vector.tensor_tensor(out=attn[:], in0=pex[:],
                                                in1=rib,
                                                op=mybir.AluOpType.mult)

                        if V_FP8:
                            vf8 = kvs[:, :, KB:BL].bitcast(F8)
                            vf = bass.AP(tensor=vf8.tensor,
                                         offset=vf8.offset,
                                         ap=[vf8.ap[0], [2 * BL, K],
                                             [D, H], [1, D]])
                        else:
                            vv = kvs[:, :, KB:BL]
                            vf = bass.AP(tensor=vv.tensor, offset=vv.offset,
                                         ap=[vv.ap[0], [BL, K],
                                             [D, H], [1, D]])
                        ab = bass.AP(tensor=attn.tensor,
                                     offset=attn[:].offset,
                                     ap=[attn[:].ap[0], [H, K],
                                         [1, H], [0, D]])
                        nc.vector.tensor_tensor(
                            out=work[:].rearrange("p s (h d) -> p s h d",
                                                  h=H),
                            in0=vf, in1=ab, op=mybir.AluOpType.mult)
                        avs = psm.tile([P, C], F32, tag="avs")
                        avr = bass.AP(tensor=work.tensor,
                                      offset=work[:].offset,
                                      ap=[work[:].ap[0], [1, C], [C, K]])
                        nc.vector.tensor_reduce(out=avs[:], in_=avr,
                                                axis=mybir.AxisListType.X,
                                                op=mybir.AluOpType.add)
                        nc.scalar.copy(out=avsb7[:, ti, :], in_=avs[:])

                    if not do_s2c:
                        continue
                    # ---- batched tail: out-proj + residual + layernorm
                    oT_ps = ps_tp.tile([P, NB2, 2, P], BF16, tag="tp")
                    for ti in range(NB2):
                        for j in range(2):
                            nc.tensor.transpose(
                                out=oT_ps[:, ti, j, :],
                                in_=avsb7[:, ti, j * P:(j + 1) * P],
                                identity=ident_bf[:])
                    oT7 = pb.tile([P, NB2, 2, P], BF16, tag="oT7")
                    nc.scalar.copy(out=oT7[:], in_=oT_ps[:])
                    ops7 = ps_mm.tile([P, NB2, C], F32, tag="mm")
                    for ti in range(NB2):
                        for j in range(2):
                            nc.tensor.matmul(ops7[:, ti, :],
                                             lhsT=oT7[:, ti, j, :],
                                             rhs=wo_sb[:, j, :],
                                             start=(j == 0), stop=(j == 1))

                    y0 = py0.tile([P, NB2, C], F32, tag="y0")
                    bo_b = bass.AP(tensor=bo_sb.tensor,
                                   offset=bo_sb[:].offset,
                                   ap=[bo_sb[:].ap[0], [0, NB2], [1, C]])
                    nc.vector.tensor_tensor(out=y0[:], in0=ops7[:], in1=bo_b,
                                            op=mybir.AluOpType.add)
                    nc.vector.tensor_tensor(out=y0[:], in0=y0[:], in1=xr7[:],
                                            op=mybir.AluOpType.add)
                    sq = py0.tile([P, NB2, C], F32, tag="sq")
                    nc.vector.tensor_tensor(out=sq[:], in0=y0[:], in1=y0[:],
                                            op=mybir.AluOpType.mult)
                    s1 = psm.tile([P, NB2], F32, tag="s1")
                    nc.vector.tensor_reduce(out=s1[:], in_=y0[:],
                                            axis=mybir.AxisListType.X,
                                            op=mybir.AluOpType.add)
                    s2r = psm.tile([P, NB2], F32, tag="s2r")
                    nc.vector.tensor_reduce(out=s2r[:], in_=sq[:],
                                            axis=mybir.AxisListType.X,
                                            op=mybir.AluOpType.add)
                    invc_b = bass.AP(tensor=invc_sb.tensor,
                                     offset=invc_sb[:].offset,
                                     ap=[invc_sb[:].ap[0], [0, NB2]])
                    mu = psm.tile([P, NB2], F32, tag="mu")
                    nc.vector.tensor_tensor(out=mu[:], in0=s1[:], in1=invc_b,
                                            op=mybir.AluOpType.mult)
                    e2 = psm.tile([P, NB2], F32, tag="e2")
                    nc.vector.tensor_tensor(out=e2[:], in0=s2r[:],
                                            in1=invc_b,
                                            op=mybir.AluOpType.mult)
                    var = psm.tile([P, NB2], F32, tag="var")
                    nc.vector.tensor_tensor(out=var[:], in0=mu[:], in1=mu[:],
                                            op=mybir.AluOpType.mult)
                    nc.vector.tensor_tensor(out=var[:], in0=e2[:],
                                            in1=var[:],
                                            op=mybir.AluOpType.subtract)
                    std = psm.tile([P, NB2], F32, tag="std")
                    nc.scalar.activation(out=std[:], in_=var[:],
                                         func=mybir.ActivationFunctionType.Sqrt,
                                         bias=eps_sb[:])
                    rstd = psm.tile([P, NB2], F32, tag="rstd")
                    nc.vector.reciprocal(out=rstd[:], in_=std[:])

                    mu_b = bass.AP(tensor=mu.tensor, offset=mu[:].offset,
                                   ap=[mu[:].ap[0], [1, NB2], [0, C]])
                    rstd_b = bass.AP(tensor=rstd.tensor,
                                     offset=rstd[:].offset,
                                     ap=[rstd[:].ap[0], [1, NB2], [0, C]])
                    gam_b = bass.AP(tensor=gam_sb.tensor,
                                    offset=gam_sb[:].offset,
                                    ap=[gam_sb[:].ap[0], [0, NB2], [1, C]])
                    bet_b = bass.AP(tensor=bet_sb.tensor,
                                    offset=bet_sb[:].offset,
                                    ap=[bet_sb[:].ap[0], [0, NB2], [1, C]])
                    nc.vector.tensor_tensor(out=y0[:], in0=y0[:], in1=mu_b,
                                            op=mybir.AluOpType.subtract)
                    nc.vector.tensor_tensor(out=y0[:], in0=y0[:], in1=rstd_b,
                                            op=mybir.AluOpType.mult)
                    nc.vector.tensor_tensor(out=y0[:], in0=y0[:], in1=gam_b,
                                            op=mybir.AluOpType.mult)
                    nc.vector.tensor_tensor(out=yb7[:], in0=y0[:], in1=bet_b,
                                            op=mybir.AluOpType.add)
                    dst = bass.AP(tensor=y.ap().tensor,
                                  offset=b * NB2 * P * C,
                                  ap=[[C, P], [P * C, NB2], [1, C]])
                    nc.sync.dma_start(out=dst, in_=yb7[:])

    nc.compile()
    return nc


# ------------------------------------------------------------------ driver

_NC_CACHE = {}
RUN_KWARGS = {}
LAST_RESULT = None


def _get_nc(cfg_key, cfg):
    if cfg_key not in _NC_CACHE:
        _NC_CACHE[cfg_key] = build_nc(cfg)
    return _NC_CACHE[cfg_key]


def kernel(**inputs):
    global LAST_RESULT
    from concourse.bass_utils import run_bass_kernel_spmd

    cfg = full_cfg()
    in_maps = host_prep(inputs, cfg)
    nc = _get_nc("full", cfg)
    res = run_bass_kernel_spmd(nc, in_maps, core_ids=list(range(NCORES)),
                               **RUN_KWARGS)
    LAST_RESULT = res
    y = np.concatenate([res.results[c]["y"] for c in range(NCORES)], 0)
    return np.ascontiguousarray(y[:N])


if __name__ == "__main__":
    pass
